# revision 1
# baseline (speedup 1.0000x reference)
"""Trainium2 Bass kernel for nn_DGEBlock (dense transformer block with
MoE-gated linears), distributed over 8 NeuronCores.

Sharding: data-parallel over batch (2 groups of 4 cores) x sequence-parallel
over tokens within each batch (512 tokens per core). Weights are replicated
(bf16, host pre-tiled); activations live feature-major ("T-layout": [d, tok])
in SBUF so projections are lhsT=W^T-tile @ rhs=activation with no activation
transposes. V is projected in token-major (N-)layout directly so attention's
PV matmuls need no transposes either. The only collectives are two 4-rank
AllGathers (V then K, bf16); their dependent loads are issued from the
GpSimd/Vector queues so they never head-of-line-block the Sync queue's
weight streaming. Output is returned token-sharded and reassembled on host.
"""

import sys

for _p in ("/opt/trn_rl_repo",):
    if _p not in sys.path:
        sys.path.append(_p)

import numpy as np
import ml_dtypes

# ---------------------------------------------------------------- constants
B = 2
T = 2048
D = 2048
H = 16
HD = 128
FF = 4 * D  # 8192
EPS = 1e-5

N_CORES = 8
GROUP = 4  # cores per batch group (sequence-parallel degree)
S = T // GROUP  # tokens per core = 512
P = 128
NT = D // P  # 16 feature tiles
NF = FF // P  # 64 hidden tiles
NKB = T // P  # 16 key blocks per batch
ISCALE = 1.0 / float(np.sqrt(HD))

RG = [[0, 1, 2, 3], [4, 5, 6, 7]]

_BF = ml_dtypes.bfloat16

_COMPILED = None


# ------------------------------------------------------------- host prep
def _w_tiled(W):
    """W [dout, din] -> [nj, 128, nt, 128] bf16 such that
    out[j, p, t, jc] == W[j*128+jc, t*128+p]  (= W^T tile (t, j)).
    Per (j, partition p) the free dims (t, jc) are contiguous in memory."""
    dout, din = W.shape
    nj, nt = dout // P, din // P
    return np.ascontiguousarray(
        W.reshape(nj, P, nt, P).transpose(0, 3, 2, 1).astype(_BF)
    )


def _b_cols(b):
    """b [dout] -> [128, nj] fp32: column j holds b[j*128:(j+1)*128]."""
    nj = b.shape[0] // P
    return np.ascontiguousarray(b.reshape(nj, P).T.astype(np.float32))


# ------------------------------------------------------------- device build
def _build():
    from concourse import bacc, tile, mybir

    fp32 = mybir.dt.float32
    bf16 = mybir.dt.bfloat16
    AF = mybir.ActivationFunctionType
    ALU = mybir.AluOpType

    nc = bacc.Bacc("TRN2", target_bir_lowering=False, debug=False,
                   num_devices=N_CORES)

    # ---- I/O tensors
    xT_d = nc.dram_tensor("xT", [D, S], fp32, kind="ExternalInput")
    wd = {}
    for nm in ("Wq", "Wgq", "Wk", "Wgk", "Wo", "Wgo"):
        wd[nm] = nc.dram_tensor(nm, [NT, P, NT, P], bf16, kind="ExternalInput")
    for nm in ("Win", "Wgin"):
        wd[nm] = nc.dram_tensor(nm, [NF, P, NT, P], bf16, kind="ExternalInput")
    for nm in ("Wout", "Wgout"):
        wd[nm] = nc.dram_tensor(nm, [NT, P, NF, P], bf16, kind="ExternalInput")
    # V projection runs in N-layout: plain W^T [din, dout] bf16 + bias rows
    wd["WvT"] = nc.dram_tensor("WvT", [D, D], bf16, kind="ExternalInput")
    wd["WgvT"] = nc.dram_tensor("WgvT", [D, D], bf16, kind="ExternalInput")
    bvrow_d = nc.dram_tensor("bvrow", [1, D], bf16, kind="ExternalInput")
    bgvrow_d = nc.dram_tensor("bgvrow", [1, D], bf16, kind="ExternalInput")
    bd = {}
    for nm in ("bq", "bgq", "bk", "bgk", "bo", "bgo",
               "bout", "bgout", "g1", "bt1", "g2", "bt2"):
        bd[nm] = nc.dram_tensor(nm, [P, NT], fp32, kind="ExternalInput")
    for nm in ("bin", "bgin"):
        bd[nm] = nc.dram_tensor(nm, [P, NF], fp32, kind="ExternalInput")
    out_d = nc.dram_tensor("outT", [D, S], fp32, kind="ExternalOutput")

    with tile.TileContext(nc) as tc:
        with (
            tc.tile_pool(name="const", bufs=1) as constp,
            tc.tile_pool(name="bias", bufs=1) as biasp,
            tc.tile_pool(name="rows", bufs=1) as rows,
            tc.tile_pool(name="dram", bufs=1, space="DRAM") as dramp,
        ):
            ones_col = constp.tile([P, 1], bf16)
            nc.vector.memset(ones_col[:], 1.0)
            ones_row = constp.tile([1, P], bf16)
            nc.vector.memset(ones_row[:], 1.0)
            eps_t = constp.tile([1, 1], fp32)
            nc.vector.memset(eps_t[:], EPS)
            bvrow = constp.tile([1, D], bf16)
            nc.sync.dma_start(bvrow[:], bvrow_d.ap())
            bgvrow = constp.tile([1, D], bf16)
            nc.sync.dma_start(bgvrow[:], bgvrow_d.ap())

            bias = {}
            for nm in bd:
                ncols = NF if nm in ("bin", "bgin") else NT
                btile = biasp.tile([P, ncols], fp32, name=f"bias_{nm}")
                nc.sync.dma_start(btile[:], bd[nm].ap())
                bias[nm] = btile

            # ---------- helpers ----------
            def ln_T(src, gname, bname, hpool, tmpool, psln, name):
                """LayerNorm over the feature dim of a T-layout activation.

                src: SBUF tile [128, NT, S] fp32 -> returns bf16 [128, NT, S].
                Stats via ones-matmuls (contract over partitions); per-token
                scale/shift rows are broadcast to [128, S] via rank-1 matmuls.
                Row chain kept on DVE (ACT only for Sqrt / Square / the final
                per-tile affine) to minimize engine hops and table reloads.
                """
                xbf = tmpool.tile([P, NT, S], bf16, name=f"{name}_xbf")
                sq = tmpool.tile([P, NT, S], bf16, name=f"{name}_sq")
                for t in range(NT):
                    nc.vector.tensor_copy(xbf[:, t, :], src[:, t, :])
                    nc.scalar.activation(sq[:, t, :], src[:, t, :], AF.Square)
                S1 = psln.tile([1, S], fp32, name=f"{name}_S1", tag="ln_S1")
                S2 = psln.tile([1, S], fp32, name=f"{name}_S2", tag="ln_S2")
                for t in range(NT):
                    nc.tensor.matmul(S1[:], ones_col[:], xbf[:, t, :],
                                     start=(t == 0), stop=(t == NT - 1))
                for t in range(NT):
                    nc.tensor.matmul(S2[:], ones_col[:], sq[:, t, :],
                                     start=(t == 0), stop=(t == NT - 1))

                def row(nm, dt=fp32):
                    return rows.tile([1, S], dt, name=f"{name}_{nm}",
                                     tag=f"ln_{nm}")

                mean = row("mean")
                nc.vector.tensor_scalar_mul(mean[:], S1[:], 1.0 / D)
                m2 = row("m2")
                nc.vector.tensor_scalar_mul(m2[:], S2[:], 1.0 / D)
                msq = row("msq")
                nc.vector.tensor_tensor(msq[:], mean[:], mean[:],
                                        op=ALU.mult)
                var = row("var")
                nc.vector.tensor_tensor(var[:], m2[:], msq[:],
                                        op=ALU.subtract)
                std = row("std")
                nc.scalar.activation(std[:], var[:], AF.Sqrt,
                                     bias=eps_t[:])
                rstd = row("rstd")
                nc.vector.reciprocal(rstd[:], std[:])
                rstd_bf = row("rstdbf", bf16)
                nc.vector.tensor_copy(rstd_bf[:], rstd[:])
                mr_bf = row("mrbf", bf16)
                nc.vector.tensor_tensor(mr_bf[:], mean[:], rstd[:],
                                        op=ALU.mult)
                Ab_p = psln.tile([P, S], fp32, name=f"{name}_Abp",
                                 tag="ln_Abp")
                nc.tensor.matmul(Ab_p[:], ones_row[:], rstd_bf[:])
                Bb_p = psln.tile([P, S], fp32, name=f"{name}_Bbp",
                                 tag="ln_Bbp")
                nc.tensor.matmul(Bb_p[:], ones_row[:], mr_bf[:])
                Ab = tmpool.tile([P, S], fp32, name=f"{name}_Ab")
                nc.vector.tensor_copy(Ab[:], Ab_p[:])
                Bb = tmpool.tile([P, S], fp32, name=f"{name}_Bb")
                nc.vector.tensor_copy(Bb[:], Bb_p[:])
                h = hpool.tile([P, NT, S], bf16, name=f"{name}_h")
                for t in range(NT):
                    tmp = tmpool.tile([P, S], fp32, name=f"{name}_t0_{t}",
                                      tag="ln_t0", bufs=3)
                    nc.vector.tensor_tensor(tmp[:], src[:, t, :], Ab[:],
                                            op=ALU.mult)
                    tmp2 = tmpool.tile([P, S], fp32, name=f"{name}_t1_{t}",
                                       tag="ln_t1", bufs=3)
                    nc.vector.tensor_tensor(tmp2[:], tmp[:], Bb[:],
                                            op=ALU.subtract)
                    nc.scalar.activation(h[:, t, :], tmp2[:], AF.Identity,
                                         bias=bias[bname][:, t:t + 1],
                                         scale=bias[gname][:, t:t + 1])
                return h

            def proj_gated(src, nt, nj, wname, wgname, bgname, wpool,
                           pspool, epilogue, tchunk=None, wbufs=3):
                """Gated projection in T-layout: for each output tile j,
                main/gate = sum_t W^T(t,j).T @ src[:,t,:], then
                epilogue(j, main_psum, sig_sbuf)."""
                if tchunk is None:
                    tchunk = nt
                nchunk = nt // tchunk
                for j in range(nj):
                    main = pspool.tile([P, S], fp32, name=f"{wname}_m{j}",
                                       tag="pj_main", bufs=2)
                    gate = pspool.tile([P, S], fp32, name=f"{wname}_g{j}",
                                       tag="pj_gate", bufs=2)
                    for ci in range(nchunk):
                        wt = wpool.tile([P, tchunk, P], bf16, tag="wmain",
                                        name=f"w_{wname}_{j}_{ci}",
                                        bufs=wbufs)
                        nc.sync.dma_start(
                            wt[:],
                            wd[wname].ap()[j, :,
                                           ci * tchunk:(ci + 1) * tchunk, :])
                        for ti in range(tchunk):
                            t = ci * tchunk + ti
                            nc.tensor.matmul(main[:], wt[:, ti, :],
                                             src[:, t, :],
                                             start=(t == 0),
                                             stop=(t == nt - 1))
                    for ci in range(nchunk):
                        wg = wpool.tile([P, tchunk, P], bf16, tag="wgate",
                                        name=f"w_{wgname}_{j}_{ci}",
                                        bufs=wbufs)
                        nc.sync.dma_start(
                            wg[:],
                            wd[wgname].ap()[j, :,
                                            ci * tchunk:(ci + 1) * tchunk, :])
                        for ti in range(tchunk):
                            t = ci * tchunk + ti
                            nc.tensor.matmul(gate[:], wg[:, ti, :],
                                             src[:, t, :],
                                             start=(t == 0),
                                             stop=(t == nt - 1))
                    sig = wpool.tile([P, S], bf16, tag="sig",
                                     name=f"sig_{wname}_{j}", bufs=3)
                    nc.scalar.activation(sig[:], gate[:], AF.Sigmoid,
                                         bias=bias[bgname][:, j:j + 1])
                    epilogue(j, main, sig)

            # x2 outlives phases A-C (used by LN2 + MLP residual)
            with tc.tile_pool(name="x2p", bufs=1) as x2p:
              with tc.tile_pool(name="xt", bufs=1) as xtp:
                xt = xtp.tile([P, NT, S], fp32)
                xT_v = xT_d.ap().rearrange("(t p) s -> t p s", p=P)
                for t in range(NT):
                    nc.sync.dma_start(xt[:, t, :], xT_v[t])

                vN_bounce = dramp.tile([S, D], bf16)
                k_bounce = dramp.tile([D, S], bf16)
                vgN = dramp.tile([GROUP * S, D], bf16)
                kg = dramp.tile([GROUP * D, S], bf16)

                with tc.tile_pool(name="yp", bufs=1) as ypool:
                  with tc.tile_pool(name="qp", bufs=1) as qpool:
                    q = qpool.tile([P, NT, S], bf16)

                    with tc.tile_pool(name="hq", bufs=1) as hqp:
                        with (
                            tc.tile_pool(name="ln1tmp", bufs=1) as ln1tmp,
                            tc.tile_pool(name="ln1ps", bufs=1,
                                         space="PSUM") as ln1ps,
                        ):
                            h1 = ln_T(xt, "g1", "bt1", hqp, ln1tmp, ln1ps,
                                      "ln1")

                        # ---- V projection, N-layout (option i) ----
                        with (
                            tc.tile_pool(name="wv", bufs=1) as wvp,
                            tc.tile_pool(name="vps", bufs=1,
                                         space="PSUM") as vps,
                        ):
                            TC = NT // 2
                            for n in range(4):
                                vmain = [vps.tile([P, S], fp32,
                                                  tag="v_main", bufs=4,
                                                  name=f"vm_{n}_{m}")
                                         for m in range(4)]
                                vgate = [vps.tile([P, S], fp32,
                                                  tag="v_gate", bufs=4,
                                                  name=f"vg_{n}_{m}")
                                         for m in range(4)]
                                for ci in range(2):
                                    wvt = wvp.tile([P, TC, 4 * P], bf16,
                                                   tag="wv", bufs=2,
                                                   name=f"wv_{n}_{ci}")
                                    wgvt = wvp.tile([P, TC, 4 * P], bf16,
                                                    tag="wgv", bufs=2,
                                                    name=f"wgv_{n}_{ci}")
                                    for ti in range(TC):
                                        t = ci * TC + ti
                                        nc.sync.dma_start(
                                            wvt[:, ti, :],
                                            wd["WvT"].ap()[t * P:(t + 1) * P,
                                                           n * S:(n + 1) * S])
                                        nc.sync.dma_start(
                                            wgvt[:, ti, :],
                                            wd["WgvT"].ap()[
                                                t * P:(t + 1) * P,
                                                n * S:(n + 1) * S])
                                    for m in range(4):
                                        for ti in range(TC):
                                            t = ci * TC + ti
                                            nc.tensor.matmul(
                                                vmain[m][:],
                                                h1[:, t, m * P:(m + 1) * P],
                                                wvt[:, ti, :],
                                                start=(t == 0), stop=False)
                                        for ti in range(TC):
                                            t = ci * TC + ti
                                            nc.tensor.matmul(
                                                vgate[m][:],
                                                h1[:, t, m * P:(m + 1) * P],
                                                wgvt[:, ti, :],
                                                start=(t == 0), stop=False)
                                for m in range(4):
                                    nc.tensor.matmul(
                                        vmain[m][:], ones_row[:],
                                        bvrow[:, n * S:(n + 1) * S],
                                        start=False, stop=True)
                                    nc.tensor.matmul(
                                        vgate[m][:], ones_row[:],
                                        bgvrow[:, n * S:(n + 1) * S],
                                        start=False, stop=True)
                                    vsig = wvp.tile([P, S], bf16,
                                                    tag="vsig", bufs=3,
                                                    name=f"vsig_{n}_{m}")
                                    nc.scalar.activation(vsig[:],
                                                         vgate[m][:],
                                                         AF.Sigmoid)
                                    vout = wvp.tile([P, S], bf16,
                                                    tag="vout", bufs=3,
                                                    name=f"vout_{n}_{m}")
                                    nc.vector.tensor_tensor(
                                        vout[:], vmain[m][:], vsig[:],
                                        op=ALU.mult)
                                    nc.scalar.dma_start(
                                        vN_bounce[m * P:(m + 1) * P,
                                                  n * S:(n + 1) * S],
                                        vout[:])

                        nc.gpsimd.collective_compute(
                            "AllGather", ALU.bypass, ins=[vN_bounce[:]],
                            outs=[vgN[:]], replica_groups=RG)

                        # ---- K projection (T-layout) + AllGather ----
                        with (
                            tc.tile_pool(name="wproj", bufs=1) as wpool,
                            tc.tile_pool(name="pjps", bufs=1,
                                         space="PSUM") as pjps,
                        ):
                            def k_epi(j, main, sig):
                                kv = wpool.tile([P, S], bf16, tag="kv_out",
                                                name=f"kv_k_{j}", bufs=3)
                                nc.vector.scalar_tensor_tensor(
                                    kv[:], main[:], bias["bk"][:, j:j + 1],
                                    sig[:], op0=ALU.add, op1=ALU.mult)
                                nc.scalar.dma_start(
                                    k_bounce[j * P:(j + 1) * P, :], kv[:])

                            proj_gated(h1, NT, NT, "Wk", "Wgk", "bgk",
                                       wpool, pjps, k_epi)

                            nc.gpsimd.collective_compute(
                                "AllGather", ALU.bypass, ins=[k_bounce[:]],
                                outs=[kg[:]], replica_groups=RG)

                            def q_epi(j, main, sig):
                                nc.vector.scalar_tensor_tensor(
                                    q[:, j, :], main[:],
                                    bias["bq"][:, j:j + 1],
                                    sig[:], op0=ALU.add, op1=ALU.mult)

                            proj_gated(h1, NT, NT, "Wq", "Wgq", "bgq",
                                       wpool, pjps, q_epi)

                    # ---- phase B: attention ----
                    with (
                        tc.tile_pool(name="vres", bufs=1) as vresp,
                        tc.tile_pool(name="kstream", bufs=2) as kpool,
                        tc.tile_pool(name="apool", bufs=4) as apool,
                        tc.tile_pool(name="atps", bufs=1,
                                     space="PSUM") as atps,
                    ):
                        y = ypool.tile([P, NT, S], bf16)
                        # V resident [k-part, kb, d]; plain loads from the
                        # gathered N-layout V, issued on the GpSimd queue.
                        Vt = vresp.tile([P, NKB, D], bf16)
                        for kb in range(NKB):
                            nc.gpsimd.dma_start(
                                Vt[:, kb, :],
                                vgN[kb * P:(kb + 1) * P, :])

                        head_state = {}

                        def finalize_head(h, Zp_h, Yp_h):
                            urow = rows.tile([1, S], fp32, name=f"u_{h}",
                                             tag="urow", bufs=2)
                            nc.vector.reciprocal(urow[:], Zp_h[:])
                            ubf = rows.tile([1, S], bf16, name=f"ubf_{h}",
                                            tag="ubf", bufs=2)
                            nc.vector.tensor_copy(ubf[:], urow[:])
                            Up = atps.tile([P, S], fp32, name=f"Up_{h}",
                                           tag="logits", bufs=4)
                            nc.tensor.matmul(Up[:], ones_row[:], ubf[:])
                            Us = apool.tile([P, S], bf16, tag="Us",
                                            name=f"Us_{h}")
                            nc.vector.tensor_copy(Us[:], Up[:])
                            nc.vector.tensor_tensor(y[:, h, :], Yp_h[:],
                                                    Us[:], op=ALU.mult)

                        for hh in range(H):
                            Kh = kpool.tile([P, NKB * P], bf16, tag="Kh",
                                            name=f"Kh_{hh}")
                            for s_ in range(GROUP):
                                nc.gpsimd.dma_start(
                                    Kh[:, s_ * S:(s_ + 1) * S],
                                    kg[s_ * D + hh * P:
                                       s_ * D + (hh + 1) * P, :])
                            Zp = atps.tile([1, S], fp32, name=f"Z_{hh}",
                                           tag="Zp", bufs=2)
                            Yp = atps.tile([P, S], fp32, name=f"Y_{hh}",
                                           tag="Yp", bufs=2)
                            ats = {}

                            def do_L(kb, hh=hh, Kh=Kh, ats=ats):
                                Lp = atps.tile([P, S], fp32,
                                               name=f"L_{hh}_{kb}",
                                               tag="logits", bufs=4)
                                nc.tensor.matmul(
                                    Lp[:], Kh[:, kb * P:(kb + 1) * P],
                                    q[:, hh, :])
                                At = apool.tile([P, S], bf16, tag="At",
                                                name=f"At_{hh}_{kb}",
                                                bufs=6)
                                nc.scalar.activation(At[:], Lp[:], AF.Exp,
                                                     scale=ISCALE)
                                ats[kb] = At

                            do_L(0)
                            do_L(1)
                            for kb in range(NKB):
                                if kb + 2 < NKB:
                                    do_L(kb + 2)
                                nc.tensor.matmul(Zp[:], ones_col[:],
                                                 ats[kb][:],
                                                 start=(kb == 0),
                                                 stop=(kb == NKB - 1))
                                nc.tensor.matmul(
                                    Yp[:],
                                    Vt[:, kb, hh * P:(hh + 1) * P],
                                    ats[kb][:],
                                    start=(kb == 0),
                                    stop=(kb == NKB - 1))
                                if kb == 3 and hh > 0:
                                    finalize_head(hh - 1,
                                                  *head_state[hh - 1])
                            head_state[hh] = (Zp, Yp)
                        finalize_head(H - 1, *head_state[H - 1])

                  # ---- phase C: o-proj + residual ----
                  x2 = x2p.tile([P, NT, S], fp32, name="x2")
                  with (
                      tc.tile_pool(name="wproj2", bufs=1) as wpool2,
                      tc.tile_pool(name="pj2ps", bufs=1,
                                   space="PSUM") as pj2ps,
                  ):
                      def o_epi(j, main, sig):
                          tmp = wpool2.tile([P, S], fp32, tag="o_tmp",
                                            name=f"o_tmp_{j}", bufs=3)
                          nc.vector.scalar_tensor_tensor(
                              tmp[:], main[:], bias["bo"][:, j:j + 1],
                              sig[:], op0=ALU.add, op1=ALU.mult)
                          nc.vector.tensor_tensor(x2[:, j, :], tmp[:],
                                                  xt[:, j, :],
                                                  op=ALU.add)

                      proj_gated(y, NT, NT, "Wo", "Wgo", "bgo",
                                 wpool2, pj2ps, o_epi)

              # ---- phase D: LN2 + MLP ----
              with tc.tile_pool(name="midp", bufs=1) as midp:
                  mid = midp.tile([P, NF, S], bf16)
                  with tc.tile_pool(name="h2p", bufs=1) as h2p:
                      with (
                          tc.tile_pool(name="ln2tmp", bufs=1) as ln2tmp,
                          tc.tile_pool(name="ln2ps", bufs=1,
                                       space="PSUM") as ln2ps,
                      ):
                          h2 = ln_T(x2, "g2", "bt2", h2p, ln2tmp, ln2ps,
                                    "ln2")

                      with (
                          tc.tile_pool(name="wmlp1", bufs=1) as wm1,
                          tc.tile_pool(name="m1ps", bufs=1,
                                       space="PSUM") as m1ps,
                      ):
                          def mid_epi(j, main, sig):
                              tmp = wm1.tile([P, S], fp32, tag="mid_tmp",
                                             name=f"mid_tmp_{j}", bufs=3)
                              nc.vector.scalar_tensor_tensor(
                                  tmp[:], main[:],
                                  bias["bin"][:, j:j + 1], sig[:],
                                  op0=ALU.add, op1=ALU.mult)
                              nc.scalar.activation(mid[:, j, :], tmp[:],
                                                   AF.Gelu)

                          proj_gated(h2, NT, NF, "Win", "Wgin", "bgin",
                                     wm1, m1ps, mid_epi)

                  with (
                      tc.tile_pool(name="wmlp2", bufs=1) as wm2,
                      tc.tile_pool(name="m2ps", bufs=1,
                                   space="PSUM") as m2ps,
                  ):
                      def out_epi(j, main, sig):
                          tmp = wm2.tile([P, S], fp32, tag="out_tmp",
                                         name=f"out_tmp_{j}", bufs=3)
                          nc.vector.scalar_tensor_tensor(
                              tmp[:], main[:], bias["bout"][:, j:j + 1],
                              sig[:], op0=ALU.add, op1=ALU.mult)
                          outf = wm2.tile([P, S], fp32, tag="out_f",
                                          name=f"out_f_{j}", bufs=3)
                          nc.vector.tensor_tensor(outf[:], tmp[:],
                                                  x2[:, j, :], op=ALU.add)
                          nc.sync.dma_start(
                              out_d.ap()[j * P:(j + 1) * P, :], outf[:])

                      proj_gated(mid, NF, NT, "Wout", "Wgout", "bgout",
                                 wm2, m2ps, out_epi, tchunk=32, wbufs=2)

    nc.compile()
    return nc


def _prep_shared_inputs(inputs):
    m = {}
    for nm, w in (("Wq", "W_q"), ("Wgq", "Wg_q"), ("Wk", "W_k"),
                  ("Wgk", "Wg_k"), ("Wo", "W_o"), ("Wgo", "Wg_o"),
                  ("Win", "W_in"), ("Wgin", "Wg_in"), ("Wout", "W_out"),
                  ("Wgout", "Wg_out")):
        m[nm] = _w_tiled(np.asarray(inputs[w]))
    m["WvT"] = np.ascontiguousarray(np.asarray(inputs["W_v"]).T.astype(_BF))
    m["WgvT"] = np.ascontiguousarray(np.asarray(inputs["Wg_v"]).T.astype(_BF))
    m["bvrow"] = np.asarray(inputs["b_v"]).astype(_BF).reshape(1, D)
    m["bgvrow"] = np.asarray(inputs["bg_v"]).astype(_BF).reshape(1, D)
    for nm, bn in (("bq", "b_q"), ("bgq", "bg_q"), ("bk", "b_k"),
                   ("bgk", "bg_k"), ("bo", "b_o"), ("bgo", "bg_o"),
                   ("bin", "b_in"), ("bgin", "bg_in"), ("bout", "b_out"),
                   ("bgout", "bg_out"), ("g1", "ln1_g"), ("bt1", "ln1_b"),
                   ("g2", "ln2_g"), ("bt2", "ln2_b")):
        m[nm] = _b_cols(np.asarray(inputs[bn]))
    return m


def _install_trace_shim():
    """Provide antenv.axon_hooks (NTFF profiling) if the image lacks it."""
    import contextlib
    import ctypes
    import types

    try:
        import antenv.axon_hooks  # noqa: F401
        return
    except ImportError:
        pass
    try:
        import antenv
    except ImportError:
        return
    so_path = "/opt/axon/libaxon_pjrt.so"
    try:
        lib = ctypes.CDLL(so_path)
    except OSError:
        return
    if not hasattr(lib, "axon_start_nrt_profile"):
        return
    lib.axon_start_nrt_profile.argtypes = [ctypes.POINTER(ctypes.c_int64),
                                           ctypes.c_size_t]
    lib.axon_start_nrt_profile.restype = ctypes.c_int64
    lib.axon_stop_nrt_profile.argtypes = [ctypes.c_char_p]
    lib.axon_stop_nrt_profile.restype = ctypes.c_int64

    @contextlib.contextmanager
    def hook(output_dir, device_ids):
        import jax

        jax.devices()
        if device_ids:
            ids = (ctypes.c_int64 * len(device_ids))(*device_ids)
            rc = lib.axon_start_nrt_profile(ids, len(device_ids))
        else:
            rc = lib.axon_start_nrt_profile(None, 0)
        if rc != 0:
            raise RuntimeError(f"axon_start_nrt_profile rc={rc}")
        try:
            yield
        finally:
            n = lib.axon_stop_nrt_profile(str(output_dir).encode())
            print(f"profile: {n} ntff file(s) in {output_dir}",
                  file=sys.stderr)

    mod = types.ModuleType("antenv.axon_hooks")
    mod.get_axon_ntff_profile_hook = lambda: hook
    mod.set_axon_ntff_profile_hook = lambda h: None
    sys.modules["antenv.axon_hooks"] = mod
    antenv.axon_hooks = mod


LAST_RESULTS = None


def kernel(_trace=False, **inputs):
    global _COMPILED, LAST_RESULTS
    from concourse import bass_utils

    if _trace:
        _install_trace_shim()

    if _COMPILED is None:
        _COMPILED = _build()
    nc = _COMPILED

    shared = _prep_shared_inputs(inputs)
    x = np.asarray(inputs["x"], dtype=np.float32)  # [B, T, D]
    in_maps = []
    for c in range(N_CORES):
        g, s = divmod(c, GROUP)
        xT_c = np.ascontiguousarray(x[g, s * S:(s + 1) * S, :].T)
        m = dict(shared)
        m["xT"] = xT_c
        in_maps.append(m)

    LAST_RESULTS = bass_utils.run_bass_kernel_spmd(
        nc, in_maps, core_ids=list(range(N_CORES)), trace=_trace)

    out = np.empty((B, T, D), dtype=np.float32)
    for c in range(N_CORES):
        g, s = divmod(c, GROUP)
        out[g, s * S:(s + 1) * S, :] = LAST_RESULTS.results[c]["outT"].T
    return out



# revision 14
# speedup vs baseline: 1.4558x; 1.4558x over previous
"""Trainium2 Bass kernel for nn_DGEBlock (dense transformer block with
MoE-gated linears), distributed over 8 NeuronCores.

v2: fp8e4 DoubleRow matmuls for QKVO (main+gate), MLP gates, and attention
PV; bf16 for MLP mains and QK.  LN gamma/beta folded into the consuming
weights host-side so LN emits only (x-mu)*rstd.  Softmax Z computed by
fp8-DoubleRow ones-matmuls; 1/Z via reciprocal_approx_fast on DVE.  GELU
as 0.5*x*(1+erf(x/sqrt2)) so MLP-in stays on the sigmoid ACT table; gate
sigmoids in the attention window use tanh (exp's table) to avoid ACT
table thrash.  Exp batched over kb pairs ([128,1024] per op).

Sharding: data-parallel over batch (2 groups of 4 cores) x sequence-
parallel within group (512 tokens/core); K then V AllGather (bf16 / fp8).
"""

import sys

for _p in ("/opt/trn_rl_repo",):
    if _p not in sys.path:
        sys.path.append(_p)

import numpy as np
import ml_dtypes

# ---------------------------------------------------------------- constants
B = 2
T = 2048
D = 2048
H = 16
HD = 128
FF = 4 * D  # 8192
EPS = 1e-5

N_CORES = 8
GROUP = 4
S = T // GROUP  # 512
P = 128
NT = D // P  # 16
NF = FF // P  # 64
NKB = T // P  # 16 key blocks per batch group
ISCALE = 1.0 / float(np.sqrt(HD))

WS = 64.0        # fp8 weight scale
DS = 1.0 / WS
VS = 4.0         # fp8 V scale
YS = 2.0         # fp8 y (attn out) scale
MS = 2.0         # fp8 mid (gelu out) scale
EC = -2.0        # exp argument shift (cancels in softmax ratio)

RG = [[0, 1, 2, 3], [4, 5, 6, 7]]

_BF = ml_dtypes.bfloat16
_F8 = ml_dtypes.float8_e4m3

_COMPILED = None


# ------------------------------------------------------------- host prep
def _w_tiled(W, dtype, scale=1.0):
    """W [dout, din] -> [nj, 128, nt, 128] such that
    out[j, p, t, jc] == W[j*128+jc, t*128+p]  (= W^T tile (t, j))."""
    dout, din = W.shape
    nj, nt = dout // P, din // P
    Wt = W.reshape(nj, P, nt, P).transpose(0, 3, 2, 1).astype(np.float32) * scale
    if dtype is _F8:
        Wt = np.clip(Wt, -240.0, 240.0)
    return np.ascontiguousarray(Wt.astype(dtype))


def _b_cols(b, scale=1.0):
    """b [dout] -> [128, nj] fp32: column j holds b[j*128:(j+1)*128]."""
    nj = b.shape[0] // P
    return np.ascontiguousarray((b * scale).reshape(nj, P).T.astype(np.float32))


# ------------------------------------------------------------- device build
def _build():
    from concourse import bacc, tile, mybir

    fp32 = mybir.dt.float32
    bf16 = mybir.dt.bfloat16
    fp8 = mybir.dt.float8e4
    AF = mybir.ActivationFunctionType
    ALU = mybir.AluOpType
    DR = mybir.MatmulPerfMode.DoubleRow

    nc = bacc.Bacc("TRN2", target_bir_lowering=False, debug=False,
                   num_devices=N_CORES)

    # ---- I/O tensors
    xT_d = nc.dram_tensor("xT", [D, S], bf16, kind="ExternalInput")
    wd = {}
    for nm in ("Wq", "Wgq", "Wk", "Wgk", "Wo", "Wgo"):
        wd[nm] = nc.dram_tensor(nm, [NT, P, NT, P], fp8, kind="ExternalInput")
    wd["Win"] = nc.dram_tensor("Win", [NF, P, NT, P], bf16,
                               kind="ExternalInput")
    wd["Wgin"] = nc.dram_tensor("Wgin", [NF, P, NT, P], fp8,
                                kind="ExternalInput")
    wd["Wout"] = nc.dram_tensor("Wout", [NT, P, NF, P], bf16,
                                kind="ExternalInput")
    wd["Wgout"] = nc.dram_tensor("Wgout", [NT, P, NF, P], fp8,
                                 kind="ExternalInput")
    wd["WvT"] = nc.dram_tensor("WvT", [D, D], fp8, kind="ExternalInput")
    wd["WgvT"] = nc.dram_tensor("WgvT", [D, D], fp8, kind="ExternalInput")
    bvrow_d = nc.dram_tensor("bvrow", [1, D], bf16, kind="ExternalInput")
    bgvrow_d = nc.dram_tensor("bgvrow", [1, D], bf16, kind="ExternalInput")
    bd = {}
    for nm in ("bq", "bgq2", "bk", "bgk", "bo", "bgo", "bout", "bgout"):
        bd[nm] = nc.dram_tensor(nm, [P, NT], fp32, kind="ExternalInput")
    for nm in ("bin", "bgin"):
        bd[nm] = nc.dram_tensor(nm, [P, NF], fp32, kind="ExternalInput")
    out_d = nc.dram_tensor("outT", [D, S], fp32, kind="ExternalOutput")

    with tile.TileContext(nc) as tc:
        with (
            tc.tile_pool(name="const", bufs=1) as constp,
            tc.tile_pool(name="bias", bufs=1) as biasp,
            tc.tile_pool(name="rows", bufs=1) as rows,
            tc.tile_pool(name="dram", bufs=1, space="DRAM") as dramp,
        ):
            ones_col = constp.tile([P, 1], bf16)
            nc.vector.memset(ones_col[:], 1.0)
            ones_row = constp.tile([1, P], bf16)
            nc.vector.memset(ones_row[:], 1.0)
            ones2_8 = constp.tile([P, 2, 16], fp8)
            nc.vector.memset(ones2_8[:], 1.0)
            eps_t = constp.tile([1, 1], fp32)
            nc.vector.memset(eps_t[:], EPS)
            ec_t = constp.tile([P, 1], fp32)
            nc.vector.memset(ec_t[:], EC)
            bvrow = constp.tile([1, D], bf16)
            nc.sync.dma_start(bvrow[:], bvrow_d.ap())
            bgvrow = constp.tile([1, D], bf16)
            nc.sync.dma_start(bgvrow[:], bgvrow_d.ap())

            bias = {}
            for nm in bd:
                ncols = NF if nm in ("bin", "bgin") else NT
                btile = biasp.tile([P, ncols], fp32, name=f"bias_{nm}")
                nc.sync.dma_start(btile[:], bd[nm].ap())
                bias[nm] = btile

            # ---------- LN helper (gamma/beta pre-folded into weights) ----
            def ln_T(src, hpool, tmpool, psln, name, out_dtypes):
                """src: SBUF [128, NT, S] bf16.  Returns z=(x-mu)*rstd in
                the dtypes listed in out_dtypes (one tile per dtype)."""
                sq = tmpool.tile([P, NT, S], bf16, name=f"{name}_sq")
                S1 = psln.tile([1, S], fp32, name=f"{name}_S1", tag="ln_S1")
                S2 = psln.tile([1, S], fp32, name=f"{name}_S2", tag="ln_S2")
                for t in range(NT):
                    nc.scalar.activation(sq[:, t, :], src[:, t, :], AF.Square)
                for t in range(NT):
                    nc.tensor.matmul(S1[:], ones_col[:], src[:, t, :],
                                     start=(t == 0), stop=(t == NT - 1))
                for t in range(NT):
                    nc.tensor.matmul(S2[:], ones_col[:], sq[:, t, :],
                                     start=(t == 0), stop=(t == NT - 1))

                def row(nm, dt=fp32):
                    return rows.tile([1, S], dt, name=f"{name}_{nm}",
                                     tag=f"ln_{nm}")

                mean = row("mean")
                nc.vector.tensor_scalar_mul(mean[:], S1[:], 1.0 / D)
                m2 = row("m2")
                nc.vector.tensor_scalar_mul(m2[:], S2[:], 1.0 / D)
                msq = row("msq")
                nc.vector.tensor_tensor(msq[:], mean[:], mean[:],
                                        op=ALU.mult)
                var = row("var")
                nc.vector.tensor_tensor(var[:], m2[:], msq[:],
                                        op=ALU.subtract)
                std = row("std")
                nc.scalar.activation(std[:], var[:], AF.Sqrt, bias=eps_t[:])
                rstd = row("rstd")
                nc.vector.reciprocal_approx_fast(rstd[:], std[:])
                rstd_bf = row("rstdbf", bf16)
                nc.gpsimd.tensor_copy(rstd_bf[:], rstd[:])
                mr_bf = row("mrbf", bf16)
                nc.vector.tensor_tensor(mr_bf[:], mean[:], rstd[:],
                                        op=ALU.mult)
                Ab_p = psln.tile([P, S], fp32, name=f"{name}_Abp",
                                 tag="ln_Abp")
                nc.tensor.matmul(Ab_p[:], ones_row[:], rstd_bf[:])
                Bb_p = psln.tile([P, S], fp32, name=f"{name}_Bbp",
                                 tag="ln_Bbp")
                nc.tensor.matmul(Bb_p[:], ones_row[:], mr_bf[:])
                Ab = tmpool.tile([P, S], bf16, name=f"{name}_Ab")
                nc.vector.tensor_copy(Ab[:], Ab_p[:])
                Bb = tmpool.tile([P, S], bf16, name=f"{name}_Bb")
                nc.vector.tensor_copy(Bb[:], Bb_p[:])
                outs = [hpool.tile([P, NT, S], dt, name=f"{name}_h{i}")
                        for i, dt in enumerate(out_dtypes)]
                for t in range(NT):
                    tmp = tmpool.tile([P, S], bf16, name=f"{name}_t0_{t}",
                                      tag="ln_t0", bufs=3)
                    nc.vector.tensor_tensor(tmp[:], src[:, t, :], Ab[:],
                                            op=ALU.mult)
                    nc.vector.scalar_tensor_tensor(outs[0][:, t, :],
                                                   tmp[:], 0.0, Bb[:],
                                                   op0=ALU.add,
                                                   op1=ALU.subtract)
                    for o in outs[1:]:
                        nc.gpsimd.tensor_copy(o[:, t, :], outs[0][:, t, :])
                return outs

            # ---------- fp8 DoubleRow gated projection (T-layout) --------
            def proj_gated8(src8, nt, nj, wname, wgname, wpool, pspool,
                            epilogue, wbufs=3):
                """main/gate psums are 64x scaled; epilogue(j, main, gate)."""
                for j in range(nj):
                    main = pspool.tile([P, S], fp32, name=f"{wname}_m{j}",
                                       tag="pj_main", bufs=2)
                    gate = pspool.tile([P, S], fp32, name=f"{wname}_g{j}",
                                       tag="pj_gate", bufs=2)
                    wt = wpool.tile([P, nt, P], fp8, tag="wmain",
                                    name=f"w_{wname}_{j}", bufs=wbufs)
                    nc.sync.dma_start(wt[:], wd[wname].ap()[j])
                    for t in range(0, nt, 2):
                        nc.tensor.matmul(main[:], wt[:, t:t + 2, :],
                                         src8[:, t:t + 2, :],
                                         start=(t == 0), stop=(t == nt - 2),
                                         perf_mode=DR)
                    wg = wpool.tile([P, nt, P], fp8, tag="wgate",
                                    name=f"w_{wgname}_{j}", bufs=wbufs)
                    nc.sync.dma_start(wg[:], wd[wgname].ap()[j])
                    for t in range(0, nt, 2):
                        nc.tensor.matmul(gate[:], wg[:, t:t + 2, :],
                                         src8[:, t:t + 2, :],
                                         start=(t == 0), stop=(t == nt - 2),
                                         perf_mode=DR)
                    epilogue(j, main, gate)

            with tc.tile_pool(name="x2p", bufs=1) as x2p:
              with tc.tile_pool(name="xt", bufs=1) as xtp:
                xbf = xtp.tile([P, NT, S], bf16)
                xT_v = xT_d.ap().rearrange("(t p) s -> t p s", p=P)
                for t in range(NT):
                    nc.sync.dma_start(xbf[:, t, :], xT_v[t])

                HD2 = D // 2
                kb_half = [dramp.tile([HD2, S], bf16, name=f"kb{i}")
                           for i in range(2)]
                kg_half = [dramp.tile([GROUP * HD2, S], bf16, name=f"kg{i}")
                           for i in range(2)]
                vb_half = [dramp.tile([S, HD2], fp8, name=f"vb{i}")
                           for i in range(2)]
                vg_half = [dramp.tile([GROUP * S, HD2], fp8, name=f"vg{i}")
                           for i in range(2)]

                with tc.tile_pool(name="yp", bufs=1) as ypool:
                  with tc.tile_pool(name="qp", bufs=1) as qpool:
                    q = qpool.tile([P, NT, S], bf16)
                    y8 = ypool.tile([P, H, S], fp8, name="y8")

                    with tc.tile_pool(name="hq", bufs=1) as hqp:
                        with (
                            tc.tile_pool(name="ln1tmp", bufs=1) as ln1tmp,
                            tc.tile_pool(name="ln1ps", bufs=1,
                                         space="PSUM") as ln1ps,
                        ):
                            (h1f8,) = ln_T(xbf, hqp, ln1tmp, ln1ps, "ln1",
                                           [fp8])

                        # ---- K projection + split AllGather (K first) ----
                        with (
                            tc.tile_pool(name="wproj", bufs=1) as wpool,
                            tc.tile_pool(name="pjps", bufs=1,
                                         space="PSUM") as pjps,
                        ):
                            def k_epi(j, main, gate):
                                sig = wpool.tile([P, S], bf16, tag="sig",
                                                 name=f"sig_k_{j}", bufs=3)
                                nc.scalar.activation(
                                    sig[:], gate[:], AF.Sigmoid,
                                    bias=bias["bgk"][:, j:j + 1], scale=DS)
                                tmpm = wpool.tile([P, S], bf16, tag="tmpm",
                                                  name=f"tmpm_k_{j}", bufs=3)
                                nc.scalar.activation(
                                    tmpm[:], main[:], AF.Identity,
                                    bias=bias["bk"][:, j:j + 1], scale=DS)
                                kv = wpool.tile([P, S], bf16, tag="kv_out",
                                                name=f"kv_k_{j}", bufs=3)
                                nc.vector.tensor_tensor(kv[:], tmpm[:],
                                                        sig[:], op=ALU.mult)
                                half, jj = divmod(j, NT // 2)
                                nc.scalar.dma_start(
                                    kb_half[half][jj * P:(jj + 1) * P, :],
                                    kv[:])
                                if j == NT // 2 - 1 or j == NT - 1:
                                    nc.gpsimd.collective_compute(
                                        "AllGather", ALU.bypass,
                                        ins=[kb_half[half][:]],
                                        outs=[kg_half[half][:]],
                                        replica_groups=RG)

                            proj_gated8(h1f8, NT, NT, "Wk", "Wgk",
                                        wpool, pjps, k_epi)

                        # ---- V projection, N-layout, fp8 DR ----
                        with (
                            tc.tile_pool(name="wv", bufs=1) as wvp,
                            tc.tile_pool(name="vps", bufs=1,
                                         space="PSUM") as vps,
                        ):
                            TC = NT // 2  # 8 k-subtiles per weight chunk
                            for n in range(4):
                                vmain = [vps.tile([P, S], fp32,
                                                  tag="v_main", bufs=4,
                                                  name=f"vm_{n}_{m}")
                                         for m in range(4)]
                                vgate = [vps.tile([P, S], fp32,
                                                  tag="v_gate", bufs=4,
                                                  name=f"vg_{n}_{m}")
                                         for m in range(4)]
                                for ci in range(2):
                                    wvt = wvp.tile([P, TC, 4 * P], fp8,
                                                   tag="wv", bufs=2,
                                                   name=f"wv_{n}_{ci}")
                                    wgvt = wvp.tile([P, TC, 4 * P], fp8,
                                                    tag="wgv", bufs=2,
                                                    name=f"wgv_{n}_{ci}")
                                    for ti in range(TC):
                                        t = ci * TC + ti
                                        nc.sync.dma_start(
                                            wvt[:, ti, :],
                                            wd["WvT"].ap()[t * P:(t + 1) * P,
                                                           n * S:(n + 1) * S])
                                        nc.sync.dma_start(
                                            wgvt[:, ti, :],
                                            wd["WgvT"].ap()[
                                                t * P:(t + 1) * P,
                                                n * S:(n + 1) * S])
                                    for m in range(4):
                                        for ti in range(0, TC, 2):
                                            t = ci * TC + ti
                                            nc.tensor.matmul(
                                                vmain[m][:],
                                                h1f8[:, t:t + 2,
                                                     m * P:(m + 1) * P],
                                                wvt[:, ti:ti + 2, :],
                                                start=(t == 0), stop=False,
                                                perf_mode=DR)
                                        for ti in range(0, TC, 2):
                                            t = ci * TC + ti
                                            nc.tensor.matmul(
                                                vgate[m][:],
                                                h1f8[:, t:t + 2,
                                                     m * P:(m + 1) * P],
                                                wgvt[:, ti:ti + 2, :],
                                                start=(t == 0), stop=False,
                                                perf_mode=DR)
                                for m in range(4):
                                    nc.tensor.matmul(
                                        vmain[m][:], ones_row[:],
                                        bvrow[:, n * S:(n + 1) * S],
                                        start=False, stop=True)
                                    nc.tensor.matmul(
                                        vgate[m][:], ones_row[:],
                                        bgvrow[:, n * S:(n + 1) * S],
                                        start=False, stop=True)
                                    vsig = wvp.tile([P, S], bf16,
                                                    tag="vsig", bufs=3,
                                                    name=f"vsig_{n}_{m}")
                                    nc.scalar.activation(vsig[:],
                                                         vgate[m][:],
                                                         AF.Sigmoid,
                                                         scale=DS)
                                    vout = wvp.tile([P, S], fp8,
                                                    tag="vout", bufs=3,
                                                    name=f"vout_{n}_{m}")
                                    nc.vector.scalar_tensor_tensor(
                                        vout[:], vmain[m][:], VS * DS,
                                        vsig[:], op0=ALU.mult,
                                        op1=ALU.mult)
                                    half = n // 2
                                    nc.scalar.dma_start(
                                        vb_half[half][
                                            m * P:(m + 1) * P,
                                            (n % 2) * S:(n % 2 + 1) * S],
                                        vout[:])
                                if n == 1 or n == 3:
                                    half = n // 2
                                    nc.gpsimd.collective_compute(
                                        "AllGather", ALU.bypass,
                                        ins=[vb_half[half][:]],
                                        outs=[vg_half[half][:]],
                                        replica_groups=RG)

                        # ---- Q projection + attention, interleaved ----
                        with (
                            tc.tile_pool(name="wq", bufs=1) as wqp,
                            tc.tile_pool(name="vres", bufs=1) as vresp,
                            tc.tile_pool(name="kstream", bufs=2) as kpool,
                            tc.tile_pool(name="apool", bufs=2) as apool,
                            tc.tile_pool(name="qps", bufs=1,
                                         space="PSUM") as qps,
                            tc.tile_pool(name="atps", bufs=1,
                                         space="PSUM") as atps,
                        ):
                          # V resident [keypart, kb, d] fp8; loads issued on
                          # the gpsimd queue, half 1 deferred (AG(V1) late)
                          Vt = vresp.tile([P, NKB, D], fp8)

                          def load_vt_half(half):
                              for kb in range(NKB):
                                  nc.gpsimd.dma_start(
                                      Vt[:, kb, half * HD2:(half + 1) * HD2],
                                      vg_half[half][kb * P:(kb + 1) * P, :])

                          load_vt_half(0)

                          def q_proj_j(j):
                            main = qps.tile([P, S], fp32, name=f"q_m{j}",
                                            tag="q_main", bufs=1)
                            gate = qps.tile([P, S], fp32, name=f"q_g{j}",
                                            tag="q_gate", bufs=1)
                            wt = wqp.tile([P, NT, P], fp8, tag="wmain",
                                          name=f"w_Wq_{j}", bufs=3)
                            nc.sync.dma_start(wt[:], wd["Wq"].ap()[j])
                            for t in range(0, NT, 2):
                                nc.tensor.matmul(main[:], wt[:, t:t + 2, :],
                                                 h1f8[:, t:t + 2, :],
                                                 start=(t == 0),
                                                 stop=(t == NT - 2),
                                                 perf_mode=DR)
                            wg = wqp.tile([P, NT, P], fp8, tag="wgate",
                                          name=f"w_Wgq_{j}", bufs=3)
                            nc.sync.dma_start(wg[:], wd["Wgq"].ap()[j])
                            for t in range(0, NT, 2):
                                nc.tensor.matmul(gate[:], wg[:, t:t + 2, :],
                                                 h1f8[:, t:t + 2, :],
                                                 start=(t == 0),
                                                 stop=(t == NT - 2),
                                                 perf_mode=DR)
                            # sigmoid via tanh (exp's ACT table):
                            # sig = 0.5 + 0.5*tanh(z/2), z = main*DS + bgq
                            th = wqp.tile([P, S], bf16, tag="q_th",
                                          name=f"q_th_{j}", bufs=3)
                            nc.scalar.activation(
                                th[:], gate[:], AF.Tanh,
                                bias=bias["bgq2"][:, j:j + 1],
                                scale=DS * 0.5)
                            u = wqp.tile([P, S], bf16, tag="q_u",
                                         name=f"q_u_{j}", bufs=3)
                            nc.vector.tensor_scalar(u[:], th[:], 0.5, 0.5,
                                                    op0=ALU.mult,
                                                    op1=ALU.add)
                            tmpm = wqp.tile([P, S], bf16, tag="q_tmpm",
                                            name=f"q_tmpm_{j}", bufs=3)
                            nc.scalar.activation(
                                tmpm[:], main[:], AF.Identity,
                                bias=bias["bq"][:, j:j + 1], scale=DS)
                            nc.vector.tensor_tensor(q[:, j, :], tmpm[:],
                                                    u[:], op=ALU.mult)

                          def attn_head(hh):
                            if hh == 2:
                                load_vt_half(1)
                            half, hl = divmod(hh, H // 2)
                            Kh = kpool.tile([P, NKB * P], bf16, tag="Kh",
                                            name=f"Kh_{hh}")
                            for s_ in range(GROUP):
                                nc.gpsimd.dma_start(
                                    Kh[:, s_ * S:(s_ + 1) * S],
                                    kg_half[half][s_ * HD2 + hl * P:
                                                  s_ * HD2 + (hl + 1) * P,
                                                  :])
                            At = apool.tile([P, NKB, S], fp8, tag="At",
                                            name=f"At_{hh}")
                            Yp = atps.tile([P, S], fp32, name=f"Y_{hh}",
                                           tag="Yp", bufs=1)
                            Zp = atps.tile([16, S], fp32, name=f"Z_{hh}",
                                           tag="Zp", bufs=1)
                            for kp in range(8):
                                kb = 2 * kp
                                Lp2 = atps.tile([P, 2, S], fp32,
                                                name=f"L_{hh}_{kp}",
                                                tag="logits", bufs=2)
                                nc.tensor.matmul(
                                    Lp2[:, 0, :],
                                    Kh[:, kb * P:(kb + 1) * P],
                                    q[:, hh, :])
                                nc.tensor.matmul(
                                    Lp2[:, 1, :],
                                    Kh[:, (kb + 1) * P:(kb + 2) * P],
                                    q[:, hh, :])
                                nc.scalar.activation(
                                    At[:, kb:kb + 2, :], Lp2[:, :, :],
                                    AF.Exp, bias=ec_t[:], scale=ISCALE)
                                nc.tensor.matmul(
                                    Yp[:], Vt[:, kb:kb + 2,
                                              hh * P:(hh + 1) * P],
                                    At[:, kb:kb + 2, :],
                                    start=(kp == 0), stop=(kp == 7),
                                    perf_mode=DR)
                                nc.tensor.matmul(
                                    Zp[:], ones2_8[:, :, :],
                                    At[:, kb:kb + 2, :],
                                    start=(kp == 0), stop=(kp == 7),
                                    perf_mode=DR)
                            urow = rows.tile([1, S], fp32, name=f"u_{hh}",
                                             tag="urow", bufs=2)
                            nc.vector.reciprocal_approx_fast(urow[:],
                                                             Zp[0:1, :])
                            ubf = rows.tile([1, S], bf16, name=f"ubf_{hh}",
                                            tag="ubf", bufs=2)
                            nc.gpsimd.tensor_copy(ubf[:], urow[:])
                            Up = atps.tile([P, 2, S], fp32, name=f"Up_{hh}",
                                           tag="logits", bufs=2)
                            nc.tensor.matmul(Up[:, 0, :], ones_row[:],
                                             ubf[:])
                            Us = kpool.tile([P, S], bf16, tag="Us",
                                            name=f"Us_{hh}")
                            nc.vector.tensor_copy(Us[:], Up[:, 0, :])
                            # y8 = fp8(YS * Yp * u / VS)
                            nc.vector.scalar_tensor_tensor(
                                y8[:, hh, :], Yp[:], YS / VS, Us[:],
                                op0=ALU.mult, op1=ALU.mult)

                          for hh in range(H):
                              q_proj_j(hh)
                              if hh >= 1:
                                  attn_head(hh - 1)
                          attn_head(H - 1)

                  # ---- o-proj + residual ----
                  x2 = x2p.tile([P, NT, S], bf16, name="x2")
                  with (
                      tc.tile_pool(name="wproj2", bufs=1) as wpool2,
                      tc.tile_pool(name="pj2ps", bufs=1,
                                   space="PSUM") as pj2ps,
                  ):
                      def o_epi(j, main, gate):
                          sig = wpool2.tile([P, S], bf16, tag="sig",
                                            name=f"sig_o_{j}", bufs=3)
                          nc.scalar.activation(
                              sig[:], gate[:], AF.Sigmoid,
                              bias=bias["bgo"][:, j:j + 1], scale=DS / YS)
                          tmpm = wpool2.tile([P, S], bf16, tag="tmpm",
                                             name=f"tmpm_o_{j}", bufs=3)
                          nc.scalar.activation(
                              tmpm[:], main[:], AF.Identity,
                              bias=bias["bo"][:, j:j + 1], scale=DS / YS)
                          yo = wpool2.tile([P, S], bf16, tag="o_yo",
                                           name=f"o_yo_{j}", bufs=3)
                          nc.vector.tensor_tensor(yo[:], tmpm[:], sig[:],
                                                  op=ALU.mult)
                          nc.vector.tensor_tensor(x2[:, j, :], yo[:],
                                                  xbf[:, j, :], op=ALU.add)

                      proj_gated8(y8, H, NT, "Wo", "Wgo", wpool2, pj2ps,
                                  o_epi)

              # ---- LN2 + MLP ----
              with tc.tile_pool(name="midp", bufs=1) as midp:
                  mid_bf = midp.tile([P, NF, S], bf16, name="mid_bf")
                  mid8 = midp.tile([P, NF, S], fp8, name="mid8")
                  with tc.tile_pool(name="h2p", bufs=1) as h2p:
                      with (
                          tc.tile_pool(name="ln2tmp", bufs=1) as ln2tmp,
                          tc.tile_pool(name="ln2ps", bufs=1,
                                       space="PSUM") as ln2ps,
                      ):
                          h2bf, h2f8 = ln_T(x2, h2p, ln2tmp, ln2ps, "ln2",
                                            [bf16, fp8])

                      with (
                          tc.tile_pool(name="wmlp1", bufs=1) as wm1,
                          tc.tile_pool(name="m1ps", bufs=1,
                                       space="PSUM") as m1ps,
                      ):
                          for j in range(NF):
                              main = m1ps.tile([P, S], fp32,
                                               name=f"in_m{j}",
                                               tag="pj_main", bufs=2)
                              gate = m1ps.tile([P, S], fp32,
                                               name=f"in_g{j}",
                                               tag="pj_gate", bufs=2)
                              wt = wm1.tile([P, NT, P], bf16, tag="wmain",
                                            name=f"w_Win_{j}", bufs=3)
                              nc.sync.dma_start(wt[:], wd["Win"].ap()[j])
                              for t in range(NT):
                                  nc.tensor.matmul(main[:], wt[:, t, :],
                                                   h2bf[:, t, :],
                                                   start=(t == 0),
                                                   stop=(t == NT - 1))
                              wg = wm1.tile([P, NT, P], fp8, tag="wgate",
                                            name=f"w_Wgin_{j}", bufs=3)
                              nc.sync.dma_start(wg[:], wd["Wgin"].ap()[j])
                              for t in range(0, NT, 2):
                                  nc.tensor.matmul(gate[:],
                                                   wg[:, t:t + 2, :],
                                                   h2f8[:, t:t + 2, :],
                                                   start=(t == 0),
                                                   stop=(t == NT - 2),
                                                   perf_mode=DR)
                              sig = wm1.tile([P, S], bf16, tag="sig",
                                             name=f"sig_in_{j}", bufs=3)
                              nc.scalar.activation(
                                  sig[:], gate[:], AF.Sigmoid,
                                  bias=bias["bgin"][:, j:j + 1], scale=DS)
                              tmpm = wm1.tile([P, S], bf16, tag="tmpm",
                                              name=f"tmpm_in_{j}", bufs=3)
                              nc.vector.scalar_tensor_tensor(
                                  tmpm[:], main[:],
                                  bias["bin"][:, j:j + 1], sig[:],
                                  op0=ALU.add, op1=ALU.mult)
                              # exact gelu: 0.5*x*(1+erf(x/sqrt2)); erf is
                              # in the sigmoid ACT table (no table reload)
                              e = wm1.tile([P, S], bf16, tag="erf",
                                           name=f"erf_{j}", bufs=3)
                              nc.scalar.activation(
                                  e[:], tmpm[:], AF.Erf,
                                  scale=0.7071067811865476)
                              u = wm1.tile([P, S], bf16, tag="gelu_u",
                                           name=f"gelu_u_{j}", bufs=3)
                              nc.vector.tensor_scalar(u[:], e[:], 0.5, 0.5,
                                                      op0=ALU.mult,
                                                      op1=ALU.add)
                              nc.vector.tensor_tensor(mid_bf[:, j, :],
                                                      tmpm[:], u[:],
                                                      op=ALU.mult)
                              nc.vector.tensor_scalar_mul(
                                  mid8[:, j, :], mid_bf[:, j, :], MS)

                  with (
                      tc.tile_pool(name="wmlp2", bufs=1) as wm2,
                      tc.tile_pool(name="m2ps", bufs=1,
                                   space="PSUM") as m2ps,
                  ):
                      TCO = 32
                      for j in range(NT):
                          main = m2ps.tile([P, S], fp32, name=f"out_m{j}",
                                           tag="pj_main", bufs=2)
                          gate = m2ps.tile([P, S], fp32, name=f"out_g{j}",
                                           tag="pj_gate", bufs=2)
                          for ci in range(2):
                              wt = wm2.tile([P, TCO, P], bf16, tag="wmain",
                                            name=f"w_Wout_{j}_{ci}",
                                            bufs=2)
                              nc.sync.dma_start(
                                  wt[:],
                                  wd["Wout"].ap()[j, :,
                                                  ci * TCO:(ci + 1) * TCO,
                                                  :])
                              for ti in range(TCO):
                                  t = ci * TCO + ti
                                  nc.tensor.matmul(main[:], wt[:, ti, :],
                                                   mid_bf[:, t, :],
                                                   start=(t == 0),
                                                   stop=(t == NF - 1))
                          for ci in range(2):
                              wg = wm2.tile([P, TCO, P], fp8, tag="wgate",
                                            name=f"w_Wgout_{j}_{ci}",
                                            bufs=2)
                              nc.sync.dma_start(
                                  wg[:],
                                  wd["Wgout"].ap()[j, :,
                                                   ci * TCO:(ci + 1) * TCO,
                                                   :])
                              for ti in range(0, TCO, 2):
                                  t = ci * TCO + ti
                                  nc.tensor.matmul(gate[:],
                                                   wg[:, ti:ti + 2, :],
                                                   mid8[:, t:t + 2, :],
                                                   start=(t == 0),
                                                   stop=(t == NF - 2),
                                                   perf_mode=DR)
                          sig = wm2.tile([P, S], bf16, tag="sig",
                                         name=f"sig_out_{j}", bufs=3)
                          nc.scalar.activation(
                              sig[:], gate[:], AF.Sigmoid,
                              bias=bias["bgout"][:, j:j + 1],
                              scale=DS / MS)
                          tmpm = wm2.tile([P, S], bf16, tag="tmpm",
                                          name=f"tmpm_out_{j}", bufs=3)
                          nc.vector.scalar_tensor_tensor(
                              tmpm[:], main[:], bias["bout"][:, j:j + 1],
                              sig[:], op0=ALU.add, op1=ALU.mult)
                          outf = wm2.tile([P, S], fp32, tag="out_f",
                                          name=f"out_f_{j}", bufs=3)
                          nc.vector.tensor_tensor(outf[:], tmpm[:],
                                                  x2[:, j, :], op=ALU.add)
                          nc.scalar.dma_start(
                              out_d.ap()[j * P:(j + 1) * P, :], outf[:])

    nc.compile()
    return nc


def _prep_shared_inputs(inputs):
    g1 = np.asarray(inputs["ln1_g"], np.float32)
    b1 = np.asarray(inputs["ln1_b"], np.float32)
    g2 = np.asarray(inputs["ln2_g"], np.float32)
    b2 = np.asarray(inputs["ln2_b"], np.float32)

    def W(nm):
        return np.asarray(inputs[nm], np.float32)

    m = {}
    # QKV / their gates: fold LN1 gamma into columns, beta into bias
    for nm, src in (("Wq", "W_q"), ("Wgq", "Wg_q"), ("Wk", "W_k"),
                    ("Wgk", "Wg_k")):
        m[nm] = _w_tiled(W(src) * g1[None, :], _F8, WS)
    for nm, src in (("Wo", "W_o"), ("Wgo", "Wg_o")):
        m[nm] = _w_tiled(W(src), _F8, WS)
    m["Win"] = _w_tiled(W("W_in") * g2[None, :], _BF)
    m["Wgin"] = _w_tiled(W("Wg_in") * g2[None, :], _F8, WS)
    m["Wout"] = _w_tiled(W("W_out"), _BF)
    m["Wgout"] = _w_tiled(W("Wg_out"), _F8, WS)
    wv = W("W_v") * g1[None, :]
    wgv = W("Wg_v") * g1[None, :]
    m["WvT"] = np.ascontiguousarray(
        np.clip(wv.T * WS, -240, 240).astype(_F8))
    m["WgvT"] = np.ascontiguousarray(
        np.clip(wgv.T * WS, -240, 240).astype(_F8))

    def bias_fold(bname, Wname, beta):
        return np.asarray(inputs[bname], np.float32) + W(Wname) @ beta

    m["bvrow"] = (bias_fold("b_v", "W_v", b1) * WS).astype(_BF).reshape(1, D)
    m["bgvrow"] = (bias_fold("bg_v", "Wg_v", b1) * WS).astype(_BF).reshape(
        1, D)
    m["bq"] = _b_cols(bias_fold("b_q", "W_q", b1))
    m["bgq2"] = _b_cols(bias_fold("bg_q", "Wg_q", b1), 0.5)
    m["bk"] = _b_cols(bias_fold("b_k", "W_k", b1))
    m["bgk"] = _b_cols(bias_fold("bg_k", "Wg_k", b1))
    m["bo"] = _b_cols(np.asarray(inputs["b_o"], np.float32))
    m["bgo"] = _b_cols(np.asarray(inputs["bg_o"], np.float32))
    m["bin"] = _b_cols(bias_fold("b_in", "W_in", b2))
    m["bgin"] = _b_cols(bias_fold("bg_in", "Wg_in", b2))
    m["bout"] = _b_cols(np.asarray(inputs["b_out"], np.float32))
    m["bgout"] = _b_cols(np.asarray(inputs["bg_out"], np.float32))
    return m


def _install_trace_shim():
    """Provide antenv.axon_hooks (NTFF profiling) if the image lacks it."""
    import contextlib
    import ctypes
    import types

    try:
        import antenv.axon_hooks  # noqa: F401
        return
    except ImportError:
        pass
    try:
        import antenv
    except ImportError:
        return
    so_path = "/opt/axon/libaxon_pjrt.so"
    try:
        lib = ctypes.CDLL(so_path)
    except OSError:
        return
    if not hasattr(lib, "axon_start_nrt_profile"):
        return
    lib.axon_start_nrt_profile.argtypes = [ctypes.POINTER(ctypes.c_int64),
                                           ctypes.c_size_t]
    lib.axon_start_nrt_profile.restype = ctypes.c_int64
    lib.axon_stop_nrt_profile.argtypes = [ctypes.c_char_p]
    lib.axon_stop_nrt_profile.restype = ctypes.c_int64

    @contextlib.contextmanager
    def hook(output_dir, device_ids):
        import jax

        jax.devices()
        if device_ids:
            ids = (ctypes.c_int64 * len(device_ids))(*device_ids)
            rc = lib.axon_start_nrt_profile(ids, len(device_ids))
        else:
            rc = lib.axon_start_nrt_profile(None, 0)
        if rc != 0:
            raise RuntimeError(f"axon_start_nrt_profile rc={rc}")
        try:
            yield
        finally:
            n = lib.axon_stop_nrt_profile(str(output_dir).encode())
            print(f"profile: {n} ntff file(s) in {output_dir}",
                  file=sys.stderr)

    mod = types.ModuleType("antenv.axon_hooks")
    mod.get_axon_ntff_profile_hook = lambda: hook
    mod.set_axon_ntff_profile_hook = lambda h: None
    sys.modules["antenv.axon_hooks"] = mod
    antenv.axon_hooks = mod


LAST_RESULTS = None


def kernel(_trace=False, **inputs):
    global _COMPILED, LAST_RESULTS
    from concourse import bass_utils

    if _trace:
        _install_trace_shim()

    if _COMPILED is None:
        _COMPILED = _build()
    nc = _COMPILED

    shared = _prep_shared_inputs(inputs)
    x = np.asarray(inputs["x"], dtype=np.float32)  # [B, T, D]
    in_maps = []
    for c in range(N_CORES):
        g, s = divmod(c, GROUP)
        xT_c = np.ascontiguousarray(x[g, s * S:(s + 1) * S, :].T.astype(_BF))
        m = dict(shared)
        m["xT"] = xT_c
        in_maps.append(m)

    LAST_RESULTS = bass_utils.run_bass_kernel_spmd(
        nc, in_maps, core_ids=list(range(N_CORES)), trace=_trace)

    out = np.empty((B, T, D), dtype=np.float32)
    for c in range(N_CORES):
        g, s = divmod(c, GROUP)
        out[g, s * S:(s + 1) * S, :] = LAST_RESULTS.results[c]["outT"].T
    return out
